# revision 17
# baseline (speedup 1.0000x reference)
"""GRU cell (single timestep) on 8 TRN2 NeuronCores, data-parallel over batch.

Contract: kernel(**inputs) takes FULL numpy inputs (as produced by the
problem's setup_inputs()) and returns the FULL (16384, 1024) float32 output.

Strategy (v3):
  - Shard batch (16384) across 8 cores -> 2048 rows/core; 4 column-blocks
    of 512. Replicate weights.
  - PE floor is 640 matmuls x 213.3ns = 136.5us (each matmul = out-free-size
    cycles at 2.4GHz regardless of dtype; fp8 DoubleRow contracts 2 k-chunks
    per instr = 2x). h-side + r/z x-side fp8 DR; hc x-side fp16 (fp8 there
    measures 2.14e-2 > the 2e-2 gate). All 16-bit stages fp16 (1.49e-2).
  - Head: the two HW DMA queues (sync, scalar; ~125 GB/s each) carry the
    critical-path tensors as fine-grained per-k-pair transfers into
    SEPARATE SBUF tiles (precise deps); the gpsimd SWDGE queue (slow
    start) carries only bias/hb/xb/late blocks. Block 0 emits r x-matmuls
    k-major and r h-matmuls in two k-passes, and z-h/hc-x with
    arrival-ordered k sequences, so the PE chases data arrival with
    minimal stalls. Warm matmuls bridge the preamble->first-data window
    and keep the PE clock ramping.
  - Tail: z-phase precomputes zh = z*h and om = 1-z so the post-tanh
    chain is 2 vector ops; the last block writes single-m outputs on the
    HW queues only (gpsimd drains early).
"""

import sys

if "/opt/trn_rl_repo" not in sys.path:
    sys.path.insert(0, "/opt/trn_rl_repo")

import numpy as np
import ml_dtypes

import concourse.bass as bass
import concourse.tile as tile
from concourse import bacc, mybir
from concourse.bass_utils import run_bass_kernel_spmd

P = 128
NCORES = 8
BATCH = 16384
NB = BATCH // NCORES          # 2048 rows per core
IN = 512
HID = 1024
KX = IN // P                  # 4
KH = HID // P                 # 8
M = HID // P                  # 8 output-feature chunks
BLK = 512                     # batch columns per compute block
NBLK = NB // BLK              # 4
WSCALE = 1024.0               # pow2 pre-scale on all weights
N_WARM = 8                    # dummy matmuls to bridge preamble -> first data

F32 = mybir.dt.float32
F16 = mybir.dt.float16
FP8 = mybir.dt.float8e4

SIG = mybir.ActivationFunctionType.Sigmoid
TANH = mybir.ActivationFunctionType.Tanh
DR = mybir.MatmulPerfMode.DoubleRow

_CACHE = {}


def _build():
    nc = bacc.Bacc("TRN2", target_bir_lowering=False, debug=False, num_devices=NCORES)

    x8d = nc.dram_tensor("x8d", [NBLK, P, KX, BLK], FP8, kind="ExternalInput").ap()
    h8d = nc.dram_tensor("h8d", [NBLK, P, KH, BLK], FP8, kind="ExternalInput").ap()
    xbd = nc.dram_tensor("xbd", [NBLK, P, 2, BLK], F16, kind="ExternalInput").ap()
    hbd = nc.dram_tensor("hbd", [NBLK, P, KH, BLK], F16, kind="ExternalInput").ap()
    wxr = nc.dram_tensor("wxr", [P, KX, HID], FP8, kind="ExternalInput").ap()
    wxz = nc.dram_tensor("wxz", [P, KX, HID], FP8, kind="ExternalInput").ap()
    wxh = nc.dram_tensor("wxh", [P, 2, HID], F16, kind="ExternalInput").ap()
    wxh8 = nc.dram_tensor("wxh8", [P, 2, HID], FP8, kind="ExternalInput").ap()
    whr = nc.dram_tensor("whr", [P, KH, HID], FP8, kind="ExternalInput").ap()
    whz = nc.dram_tensor("whz", [P, KH, HID], FP8, kind="ExternalInput").ap()
    whh = nc.dram_tensor("whh", [P, KH, HID], FP8, kind="ExternalInput").ap()
    bias = nc.dram_tensor("bias", [P, 24], F32, kind="ExternalInput").ap()
    outT = nc.dram_tensor("outT", [NBLK, P, M, BLK], F16, kind="ExternalOutput").ap()

    inv_s = 1.0 / WSCALE

    with tile.TileContext(nc) as tc:
        with (
            tc.tile_pool(name="wpool", bufs=1) as wpool,
            tc.tile_pool(name="x8pool", bufs=3) as x8pool,
            tc.tile_pool(name="h8pool", bufs=3) as h8pool,
            tc.tile_pool(name="xbpool", bufs=3) as xbpool,
            tc.tile_pool(name="hbpool", bufs=3) as hbpool,
            tc.tile_pool(name="rpool", bufs=2) as rpool,
            tc.tile_pool(name="rhpool", bufs=2) as rhpool,
            tc.tile_pool(name="zhpool", bufs=2) as zhpool,
            tc.tile_pool(name="ompool", bufs=2) as ompool,
            tc.tile_pool(name="hcpool", bufs=2) as hcpool,
            tc.tile_pool(name="opool", bufs=4) as opool,
            tc.tile_pool(name="obpool", bufs=2) as obpool,
            tc.tile_pool(name="psum", bufs=8, space=bass.MemorySpace.PSUM) as psum,
        ):
            b_s = wpool.tile([P, 24], F32)
            # half-split weight tiles so DMA deps are half-granular
            wxr_t = [wpool.tile([P, 2, HID], FP8, name=f"wxr{i}") for i in range(2)]
            wxz_t = [wpool.tile([P, 2, HID], FP8, name=f"wxz{i}") for i in range(2)]
            wxh16_s = wpool.tile([P, 2, HID], F16)
            wxh8_s = wpool.tile([P, 2, HID], FP8)
            whr_t = [wpool.tile([P, 2, HID], FP8, name=f"whr{i}") for i in range(4)]
            whz_t = [wpool.tile([P, 4, HID], FP8, name=f"whz{i}") for i in range(2)]
            whh_t = [wpool.tile([P, 4, HID], FP8, name=f"whh{i}") for i in range(2)]
            x80 = [wpool.tile([P, 2, BLK], FP8, name=f"x80{i}") for i in range(2)]
            dummy = wpool.tile([P, BLK], F16)

            x8 = [None] * NBLK   # [P, 4, BLK] fp8 (blocks 1-3)
            h8 = [None] * NBLK   # [P, 8, BLK] fp8
            xb = [None] * NBLK   # [P, 4, BLK] fp16
            hb = [None] * NBLK   # [P, 8, BLK] fp16 (blocks 1-3)

            # PE warmup: keep the PE clock ramping while the first DMAs land.
            nc.vector.memset(dummy[:], 0.0)
            warm_ps = psum.tile([P, BLK], F32, tag="ps", name="ps")
            for _ in range(N_WARM):
                nc.tensor.matmul(
                    warm_ps[:], dummy[:, 0:P], dummy[:], start=True, stop=True,
                )

            # ---- critical transfers. Scalar (A) issues only 5 early
            # transfers so its in-order queue reaches the activations
            # fast; sync (S) carries the rest of the PE-gating stream;
            # gpsimd (G, slow SWDGE) carries bias + elementwise inputs.
            # 14 head dma_starts total keeps semaphore-pool reuse stale.
            S, A, G = nc.sync, nc.scalar, nc.gpsimd
            S.dma_start(wxr_t[0][:], wxr[:, 0:2, :])
            A.dma_start(x80[0][:], x8d[0, :, 0:2, :])
            A.dma_start(wxr_t[1][:], wxr[:, 2:4, :])
            A.dma_start(x80[1][:], x8d[0, :, 2:4, :])
            h8[0] = h8pool.tile([P, KH, BLK], FP8, tag="h8", name="h8")
            S.dma_start(h8[0][:], h8d[0, :, :, :])
            A.dma_start(whr_t[0][:], whr[:, 0:2, :])
            S.dma_start(whr_t[2][:], whr[:, 4:6, :])
            S.dma_start(whr_t[3][:], whr[:, 6:8, :])
            A.dma_start(wxz_t[0][:], wxz[:, 0:2, :])
            A.dma_start(wxz_t[1][:], wxz[:, 2:4, :])
            S.dma_start(whz_t[0][:], whz[:, 0:4, :])
            S.dma_start(whz_t[1][:], whz[:, 4:8, :])
            S.dma_start(whh_t[0][:], whh[:, 0:4, :])
            S.dma_start(whh_t[1][:], whh[:, 4:8, :])
            G.dma_start(b_s[:], bias[:])
            G.dma_start(whr_t[1][:], whr[:, 2:4, :])
            hb[0] = hbpool.tile([P, KH, BLK], F16, tag="hb", name="hb")
            G.dma_start(hb[0][:], hbd[0, :, :, :])
            xb[0] = xbpool.tile([P, 2, BLK], F16, tag="xb", name="xb")
            G.dma_start(xb[0][:], xbd[0, :, :, :])
            G.dma_start(wxh16_s[:], wxh[:])
            G.dma_start(wxh8_s[:], wxh8[:])

            def fetch_block(blk):
                # x8/h8 on the sync HW queue (gate the block's matmuls);
                # hb/xb on G. The scalar engine issues nothing mid-run so
                # its queue is always free for activations.
                x8[blk] = x8pool.tile([P, KX, BLK], FP8, tag="x8", name="x8")
                S.dma_start(x8[blk][:], x8d[blk, :, :, :])
                h8[blk] = h8pool.tile([P, KH, BLK], FP8, tag="h8", name="h8")
                S.dma_start(h8[blk][:], h8d[blk, :, :, :])
                hb[blk] = hbpool.tile([P, KH, BLK], F16, tag="hb", name="hb")
                G.dma_start(hb[blk][:], hbd[blk, :, :, :])
                xb[blk] = xbpool.tile([P, 2, BLK], F16, tag="xb", name="xb")
                G.dma_start(xb[blk][:], xbd[blk, :, :, :])

            fetch_block(1)

            def x8pair(blk, j):
                return (x80[j][:, :, :] if blk == 0
                        else x8[blk][:, 2 * j : 2 * j + 2, :])

            def x_mms8(ps, wt, m, blk, only_j=None):
                # fp8 DoubleRow x-side: wt = two [P,2,HID] half tiles
                mo = bass.ts(m, P)
                js = (0, 1) if only_j is None else (only_j,)
                for j in js:
                    nc.tensor.matmul(
                        ps[:], wt[j][:, :, mo], x8pair(blk, j),
                        start=(j == 0), stop=False, perf_mode=DR,
                    )

            def x_mms_hc(ps, m, blk):
                # hc x-side hybrid: chunks 0-1 as one fp8 DoubleRow matmul
                # (reuses the x8 tile), chunks 2-3 in fp16 (rel err 1.84e-2
                # vs the 2e-2 gate; full fp8 would be 2.14e-2)
                mo = bass.ts(m, P)
                for k in range(2):
                    nc.tensor.matmul(
                        ps[:], wxh16_s[:, k, mo], xb[blk][:, k, :],
                        start=(k == 0), stop=False,
                    )
                nc.tensor.matmul(
                    ps[:], wxh8_s[:, :, mo], x8pair(blk, 0),
                    start=False, stop=False, perf_mode=DR,
                )

            def h_mms(ps, wt, m, rhs8, j_order=(0, 1, 2, 3)):
                # fp8 DoubleRow h-side; wt = 2 half or 4 quarter tiles
                mo = bass.ts(m, P)
                for i, j in enumerate(j_order):
                    if len(wt) == 4:
                        w = wt[j][:, :, mo]
                    else:
                        w = wt[j // 2][:, 2 * (j % 2) : 2 * (j % 2) + 2, mo]
                    nc.tensor.matmul(
                        ps[:], w, rhs8[:, 2 * j : 2 * j + 2, :],
                        start=False, stop=(i == 3), perf_mode=DR,
                    )

            out_eng3 = [S, G, A]
            out_eng2 = [S, A]

            for blk in range(NBLK):
                # ---- R phase: r = sigmoid(x@Wxr + bxr + h@Whr); rh8 = fp8(r*h)
                rh8 = rhpool.tile([P, KH, BLK], FP8, tag="rh8")
                ps_r = []
                if blk == 0:
                    # k-major x-passes: the first 8 matmuls need only the
                    # first x8/wxr k-pair; the j1 pass chases the next
                    # transfers; h-matmuls follow m-major
                    for m in range(M):
                        ps_r.append(psum.tile([P, BLK], F32, tag="ps", name="ps"))
                    for j in range(2):
                        for m in range(M):
                            x_mms8(ps_r[m], wxr_t, m, blk, only_j=j)
                    for m in range(M):
                        h_mms(ps_r[m], whr_t, m, h8[blk])
                else:
                    for m in range(M):
                        ps = psum.tile([P, BLK], F32, tag="ps", name="ps")
                        ps_r.append(ps)
                        x_mms8(ps, wxr_t, m, blk)
                        h_mms(ps, whr_t, m, h8[blk])
                for m in range(M):
                    rt = rpool.tile([P, BLK], F16, tag="rt")
                    nc.scalar.activation(
                        rt[:], ps_r[m][:], SIG,
                        bias=b_s[:, m : m + 1], scale=inv_s,
                    )
                    nc.vector.tensor_mul(rh8[:, m, :], rt[:], hb[blk][:, m, :])

                # ---- Z phase: z = sigmoid(x@Wxz + bxz + h@Whz)
                #      zh = z*h; om = 1-z
                zh = zhpool.tile([P, M, BLK], F16, tag="zh")
                om = ompool.tile([P, M, BLK], F16, tag="om")
                ps_z = []
                for m in range(M):
                    ps = psum.tile([P, BLK], F32, tag="ps", name="ps")
                    ps_z.append(ps)
                    x_mms8(ps, wxz_t, m, blk)
                    h_mms(ps, whz_t, m, h8[blk])
                for m in range(M):
                    zt = rpool.tile([P, BLK], F16, tag="rt")
                    nc.scalar.activation(
                        zt[:], ps_z[m][:], SIG,
                        bias=b_s[:, 8 + m : 9 + m], scale=inv_s,
                    )
                    nc.vector.tensor_mul(zh[:, m, :], zt[:], hb[blk][:, m, :])
                    nc.vector.tensor_scalar(
                        out=om[:, m, :], in0=zt[:],
                        scalar1=-1.0, scalar2=1.0,
                        op0=mybir.AluOpType.mult, op1=mybir.AluOpType.add,
                    )

                if blk + 2 < NBLK:
                    fetch_block(blk + 2)

                # ---- HC phase: hc = tanh(x@Wxh + bxh + rh@Whh)
                #      out = zh + om*hc
                last = blk == NBLK - 1
                ob = obpool.tile([P, M, BLK], F16, tag="ob")
                for m in range(M):
                    ps = psum.tile([P, BLK], F32, tag="ps", name="ps")
                    x_mms_hc(ps, m, blk)
                    h_mms(ps, whh_t, m, rh8)
                    hct = hcpool.tile([P, BLK], F16, tag="hct")
                    nc.scalar.activation(
                        hct[:], ps[:], TANH,
                        bias=b_s[:, 16 + m : 17 + m], scale=inv_s,
                    )
                    ot = opool.tile([P, BLK], F16, tag="ot")
                    nc.vector.tensor_mul(ot[:], om[:, m, :], hct[:])
                    nc.vector.tensor_add(ob[:, m, :], ot[:], zh[:, m, :])
                    if last:
                        # singles; even m on gpsimd (drains before the m7
                        # tail), odd m on sync
                        (G if m % 2 == 0 else S).dma_start(
                            outT[blk, :, m : m + 1, :], ob[:, m : m + 1, :]
                        )
                    elif m % 2 == 1:
                        (S if (m // 2) % 2 == 0 else G).dma_start(
                            outT[blk, :, m - 1 : m + 1, :], ob[:, m - 1 : m + 1, :]
                        )

    nc.compile()
    return nc


def _pack_feature_major(a: np.ndarray, nchunks: int, dtype) -> np.ndarray:
    # [rows, cols] -> [128, nchunks, cols] with [p, k, c] = a[128k+p, c]
    rows, cols = a.shape
    assert rows == nchunks * P
    return np.ascontiguousarray(
        a.reshape(nchunks, P, cols).transpose(1, 0, 2)
    ).astype(dtype)


def _block_major(a: np.ndarray, dtype) -> np.ndarray:
    # [128, K, NB] -> [NBLK, 128, K, BLK]
    p, k, nb = a.shape
    return np.ascontiguousarray(
        a.reshape(p, k, NBLK, BLK).transpose(2, 0, 1, 3).astype(dtype)
    )


def _pack_inputs(x, hidden, Wxr, bxr, Whr, Wxz, bxz, Whz, Wxh, bxh, Whh):
    f16 = np.float16
    f8 = ml_dtypes.float8_e4m3  # TRN-compatible e4m3 (max 240)
    wxr_p = _pack_feature_major(np.asarray(Wxr, np.float32) * WSCALE, KX, f8)
    wxz_p = _pack_feature_major(np.asarray(Wxz, np.float32) * WSCALE, KX, f8)
    wxh_pf = _pack_feature_major(np.asarray(Wxh, np.float32) * WSCALE, KX, np.float32)
    wxh_p = np.ascontiguousarray(wxh_pf[:, 2:4]).astype(f16)
    wxh8_p = np.ascontiguousarray(wxh_pf[:, 0:2]).astype(f8)
    whr_p = _pack_feature_major(np.asarray(Whr, np.float32) * WSCALE, KH, f8)
    whz_p = _pack_feature_major(np.asarray(Whz, np.float32) * WSCALE, KH, f8)
    whh_p = _pack_feature_major(np.asarray(Whh, np.float32) * WSCALE, KH, f8)
    bias_p = np.ascontiguousarray(
        np.concatenate(
            [
                np.asarray(b, np.float32).reshape(M, P).T
                for b in (bxr, bxz, bxh)
            ],
            axis=1,
        )
    )  # [128, 24]

    x = np.asarray(x, np.float32)
    hidden = np.asarray(hidden, np.float32)

    in_maps = []
    for c in range(NCORES):
        rows = slice(c * NB, (c + 1) * NB)
        xp = _pack_feature_major(x[rows].T, KX, np.float32)   # [128,4,2048]
        hp = _pack_feature_major(hidden[rows].T, KH, np.float32)
        in_maps.append(
            {
                "x8d": _block_major(xp, f8),
                "h8d": _block_major(hp, f8),
                "xbd": _block_major(np.ascontiguousarray(xp[:, 2:4]), f16),
                "hbd": _block_major(hp, f16),
                "wxr": wxr_p,
                "wxz": wxz_p,
                "wxh": wxh_p,
                "wxh8": wxh8_p,
                "whr": whr_p,
                "whz": whz_p,
                "whh": whh_p,
                "bias": bias_p,
            }
        )
    return in_maps


def kernel(x, hidden, Wxr, bxr, Whr, Wxz, bxz, Whz, Wxh, bxh, Whh):
    if "nc" not in _CACHE:
        _CACHE["nc"] = _build()
    nc = _CACHE["nc"]

    in_maps = _pack_inputs(
        x, hidden, Wxr, bxr, Whr, Wxz, bxz, Whz, Wxh, bxh, Whh
    )
    res = run_bass_kernel_spmd(nc, in_maps, core_ids=list(range(NCORES)))

    out = np.empty((BATCH, HID), np.float32)
    for c in range(NCORES):
        oT = np.asarray(res.results[c]["outT"], dtype=np.float32)  # [4,128,8,512]
        out[c * NB : (c + 1) * NB] = (
            oT.transpose(2, 1, 0, 3).reshape(HID, NB).T
        )
    return out
